# revision 15
# baseline (speedup 1.0000x reference)
"""Distributed Bass attention kernel for 8 TRN2 NeuronCores.

Problem: nn_Attention (B=2, NQ=512, NCTX=16384, QDIM=CDIM=512, H=8, D=64).

Sharding (per spec hint): data parallel on batch (2) x tensor parallel on
heads (4 groups of 2 heads) = 8 cores. Core i handles batch i//4, heads
[2*(i%4), 2*(i%4)+1]. Each core computes its head-slice of the attention
output, the 4 cores of a batch group AllGather the slices, and every core
runs the (tiny) output projection; the host reads it back from one core
per batch.

Device-side layout choices:
  - x and context are passed pre-transposed ([d_model, seq]) so the
    contraction dim lands on SBUF partitions without on-chip transposes.
  - scores are computed transposed (simT[j, i]) so the context mask is a
    per-partition bias of the Exp activation and the softmax denominator
    falls out of the AV matmul via an extra ones-column in V.
  - all heavy matmuls run in bf16 (fp32 is 4x slower on the PE); softmax
    accumulation stays fp32 in PSUM.
"""
import sys

sys.path.insert(0, '/opt/trn_rl_repo')

import numpy as np

import concourse.bacc as bacc
import concourse.mybir as mybir
import concourse.tile as tile
from concourse.bass_utils import run_bass_kernel_spmd

F32 = mybir.dt.float32
BF16 = mybir.dt.bfloat16
U8 = mybir.dt.uint8
AF = mybir.ActivationFunctionType
ALU = mybir.AluOpType

B = 2
NQ = 512          # query tokens (i)
NCTX = 16384      # context tokens (j)
DM = 512          # model dim
HEADS = 8
DH = 64
INNER = 512
N_CORES = 8

KC = 4              # d_model chunks of 128
NJT = NCTX // 128   # 128 j-tiles
JCH = 512           # context j-chunk per DMA (1 MiB fp32 source)
NCH = NCTX // JCH
VT = JCH // 128     # v tiles per j-chunk
SCALE = DH ** -0.5
MASK_BIG = 30000.0


def build_nc():
    nc = bacc.Bacc(None, target_bir_lowering=False, debug=False, num_devices=N_CORES)

    xt_d = nc.dram_tensor("xT", [DM, NQ], F32, kind="ExternalInput")
    ctxt_d = nc.dram_tensor("ctxT", [DM, NCTX], F32, kind="ExternalInput")
    msk_d = nc.dram_tensor("maskt", [128, NJT], U8, kind="ExternalInput")
    wq_d = nc.dram_tensor("wq", [DM, 128], F32, kind="ExternalInput")
    wk_d = nc.dram_tensor("wk", [DM, 128], F32, kind="ExternalInput")
    wv_d = nc.dram_tensor("wv", [DM, 128], F32, kind="ExternalInput")
    wout_d = nc.dram_tensor("wout", [INNER, INNER], F32, kind="ExternalInput")
    bout_d = nc.dram_tensor("boutr", [128, 4], F32, kind="ExternalInput")
    out_d = nc.dram_tensor("outT", [INNER, NQ], F32, kind="ExternalOutput")

    with tile.TileContext(nc) as tc:
        with (
            tc.tile_pool(name="const", bufs=1) as cpool,
            tc.tile_pool(name="big", bufs=1) as big,
            tc.tile_pool(name="ctx", bufs=4) as ctxpool,
            tc.tile_pool(name="pt", bufs=3) as ptpool,
            tc.tile_pool(name="fin", bufs=2) as fin,
            tc.tile_pool(name="ps", bufs=3, space="PSUM") as pps,
            tc.tile_pool(name="av", bufs=1, space="PSUM") as pav,
            tc.tile_pool(name="dram", bufs=1, space="DRAM") as dram,
        ):
            # ---- weights / x / mask: fp32 staging on the HWDGE queue, DVE cast.
            # (keeps the gpsimd SWDGE queue free for the big context stream)
            wq_f = cpool.tile([128, KC, 128], F32)
            wk_f = cpool.tile([128, KC, 128], F32)
            wv_f = cpool.tile([128, KC, 128], F32)
            xt_f = cpool.tile([128, KC, NQ], F32)
            wout_f = cpool.tile([128, KC, INNER], F32)
            wq_bf = cpool.tile([128, KC, 128], BF16)
            wk_bf = cpool.tile([128, KC, 128], BF16)
            wv_bf = cpool.tile([128, KC, 128], BF16)
            xt_bf = cpool.tile([128, KC, NQ], BF16)
            wout_bf = cpool.tile([128, KC, INNER], BF16)
            for dst, stage, src in ((wq_bf, wq_f, wq_d), (wk_bf, wk_f, wk_d),
                                    (wv_bf, wv_f, wv_d), (xt_bf, xt_f, xt_d),
                                    (wout_bf, wout_f, wout_d)):
                nc.sync.dma_start(
                    out=stage[:], in_=src.ap().rearrange("(k p) n -> p k n", p=128))
                nc.vector.tensor_copy(dst[:], stage[:])

            msk_u8 = cpool.tile([128, NJT], U8)
            nc.sync.dma_start(out=msk_u8[:], in_=msk_d[:, :])
            msk_f = cpool.tile([128, NJT], F32)
            nc.vector.tensor_copy(msk_f[:], msk_u8[:])
            bias_sb = cpool.tile([128, NJT], F32)
            nc.vector.tensor_scalar(bias_sb[:], msk_f[:], MASK_BIG, -MASK_BIG,
                                    ALU.mult, ALU.add)

            bout_sb = cpool.tile([128, 4], F32)
            nc.sync.dma_start(out=bout_sb[:], in_=bout_d[:, :])

            # ones row lives at partition 64 so it can pair with row-64 slices
            ones_sb = cpool.tile([65, 65], F32)
            nc.vector.memset(ones_sb[64:65, :], 1.0)

            # ---- qT: [128 (2h x 64d), NQ] bf16 ----
            psum_q = pps.tile([128, NQ], F32, tag="ps")
            for k in range(KC):
                nc.tensor.matmul(psum_q[:], wq_bf[:, k, :], xt_bf[:, k, :],
                                 start=(k == 0), stop=(k == KC - 1))
            qt_bf = cpool.tile([128, NQ], BF16)
            nc.vector.tensor_copy(qt_bf[:], psum_q[:])

            # ---- kT [128 (2h x 64d), NCTX] and v_aug [128 (j), NJT, 130] ----
            kt_bf = big.tile([128, NCTX], BF16)
            # per j-tile: [vA(64) | onesA | vB(64) | onesB] so the AV matmul's
            # ones-column accumulates the softmax denominator into row 64
            v_bf = big.tile([128, NJT, 130], BF16)
            nc.vector.memset(v_bf[:, :, 64:65], 1.0)
            nc.vector.memset(v_bf[:, :, 129:130], 1.0)

            for c in range(NCH):
                j0 = c * JCH
                ctx_bf = ctxpool.tile([128, KC, JCH], BF16, tag="ctx")
                nc.gpsimd.dma_start(
                    out=ctx_bf[:],
                    in_=ctxt_d.ap()[:, j0:j0 + JCH].rearrange("(k p) j -> p k j", p=128))
                psum_kt = pps.tile([128, JCH], F32, tag="ps")
                for k in range(KC):
                    nc.tensor.matmul(psum_kt[:], wk_bf[:, k, :], ctx_bf[:, k, :],
                                     start=(k == 0), stop=(k == KC - 1))
                nc.vector.tensor_copy(kt_bf[:, j0:j0 + JCH], psum_kt[:])
                for t in range(VT):
                    jt = c * VT + t
                    psum_v = pps.tile([128, 128], F32, tag="ps")
                    for k in range(KC):
                        nc.tensor.matmul(psum_v[:], ctx_bf[:, k, t * 128:(t + 1) * 128],
                                         wv_bf[:, k, :], start=(k == 0), stop=(k == KC - 1))
                    nc.vector.tensor_copy(v_bf[:, jt, 0:64], psum_v[:, 0:64])
                    nc.vector.tensor_copy(v_bf[:, jt, 65:129], psum_v[:, 64:128])

            # ---- attention: simT -> exp -> AV accumulate ----
            psum_av = [pav.tile([65, NQ], F32, tag=f"av{h}", name=f"psum_av{h}")
                       for h in range(2)]
            for t in range(NJT):
                psum_s = pps.tile([128, 2 * NQ], F32, tag="ps")
                for h in range(2):
                    nc.tensor.matmul(psum_s[:, h * NQ:(h + 1) * NQ],
                                     kt_bf[h * 64:(h + 1) * 64, t * 128:(t + 1) * 128],
                                     qt_bf[h * 64:(h + 1) * 64, :],
                                     start=True, stop=True)
                pt_bf = ptpool.tile([128, 2 * NQ], BF16, tag="pt")
                nc.scalar.activation(pt_bf[:], psum_s[:], AF.Exp,
                                     bias=bias_sb[:, t:t + 1], scale=SCALE)
                for h in range(2):
                    nc.tensor.matmul(psum_av[h][:],
                                     v_bf[:, t, h * 65:(h + 1) * 65],
                                     pt_bf[:, h * NQ:(h + 1) * NQ],
                                     start=(t == 0), stop=(t == NJT - 1),
                                     skip_group_check=True)

            # ---- normalize by the softmax denominator, stage for AllGather ----
            ag_in = dram.tile([128, NQ], BF16)
            ag_out = dram.tile([INNER, NQ], BF16)
            for h in range(2):
                linv = fin.tile([65, NQ], F32, tag="linv")
                nc.vector.reciprocal(linv[64:65, :], psum_av[h][64:65, :])
                psum_lb = pps.tile([65, NQ], F32, tag="ps")
                nc.tensor.matmul(psum_lb[:], ones_sb[64:65, :], linv[64:65, :],
                                 start=True, stop=True)
                linvb = fin.tile([65, NQ], F32, tag="linvb")
                nc.vector.tensor_copy(linvb[:], psum_lb[:])
                avn = fin.tile([64, NQ], BF16, tag="avn")
                nc.vector.tensor_tensor(avn[:], psum_av[h][0:64, :],
                                        linvb[0:64, :], ALU.mult)
                nc.sync.dma_start(out=ag_in[h * 64:(h + 1) * 64, :], in_=avn[:])

            nc.gpsimd.collective_compute(
                "AllGather", ALU.bypass,
                replica_groups=[[0, 1, 2, 3], [4, 5, 6, 7]],
                ins=[ag_in[:].opt()],
                outs=[ag_out[:].opt()],
            )

            # ---- output projection: outT = Wout^T @ attnoutT (+ bout) ----
            att_bf = cpool.tile([128, KC, NQ], BF16)
            nc.sync.dma_start(
                out=att_bf[:], in_=ag_out[:].rearrange("(k p) n -> p k n", p=128))
            for m in range(4):
                psum_o = pps.tile([128, NQ], F32, tag="ps")
                for k in range(KC):
                    nc.tensor.matmul(psum_o[:], wout_bf[:, k, m * 128:(m + 1) * 128],
                                     att_bf[:, k, :], start=(k == 0), stop=(k == KC - 1))
                out_sb = fin.tile([128, NQ], F32, tag="out")
                nc.scalar.activation(out_sb[:], psum_o[:], AF.Identity,
                                     bias=bout_sb[:, m:m + 1])
                nc.sync.dma_start(out=out_d[m * 128:(m + 1) * 128, :], in_=out_sb[:])

    nc.compile()
    return nc


_NC = None


def _get_nc():
    global _NC
    if _NC is None:
        _NC = build_nc()
    return _NC


def kernel(x, context, mask, Wq, Wkv, Wout, bout):
    x = np.asarray(x, dtype=np.float32)
    context = np.asarray(context, dtype=np.float32)
    mask = np.asarray(mask)
    Wq = np.asarray(Wq, dtype=np.float32)
    Wkv = np.asarray(Wkv, dtype=np.float32)
    Wout = np.asarray(Wout, dtype=np.float32)
    bout = np.asarray(bout, dtype=np.float32)

    nc = _get_nc()
    in_maps = []
    for core in range(N_CORES):
        b, hg = core // 4, core % 4
        cs = slice(hg * 128, (hg + 1) * 128)
        in_maps.append({
            "xT": np.ascontiguousarray(x[b].T),
            "ctxT": np.ascontiguousarray(context[b].T),
            "maskt": np.ascontiguousarray(
                mask[b].reshape(NJT, 128).T.astype(np.uint8)),
            "wq": np.ascontiguousarray(Wq[:, cs]),
            "wk": np.ascontiguousarray(Wkv[:, :INNER][:, cs]),
            "wv": np.ascontiguousarray(Wkv[:, INNER:][:, cs]),
            "wout": Wout,
            "boutr": np.ascontiguousarray(bout.reshape(4, 128).T),
        })

    res = run_bass_kernel_spmd(nc, in_maps, list(range(N_CORES)))
    out = np.empty((B, NQ, INNER), dtype=np.float32)
    for b in range(B):
        out[b] = res.results[4 * b]["outT"].T
    return out


# revision 16
# speedup vs baseline: 1.0205x; 1.0205x over previous
"""Distributed Bass attention kernel for 8 TRN2 NeuronCores.

Problem: nn_Attention (B=2, NQ=512, NCTX=16384, QDIM=CDIM=512, H=8, D=64).

Sharding (per spec hint): data parallel on batch (2) x tensor parallel on
heads (4 groups of 2 heads) = 8 cores. Core i handles batch i//4, heads
[2*(i%4), 2*(i%4)+1]. Each core computes its head-slice of the attention
output, the 4 cores of a batch group AllGather the slices, and every core
runs the (tiny) output projection; the host reads it back from one core
per batch.

Device-side layout choices:
  - x and context are passed pre-transposed ([d_model, seq]) so the
    contraction dim lands on SBUF partitions without on-chip transposes.
  - scores are computed transposed (simT[j, i]) so the context mask is a
    per-partition bias of the Exp activation and the softmax denominator
    falls out of the AV matmul via an extra ones-column in V.
  - all heavy matmuls run in bf16 (fp32 is 4x slower on the PE); softmax
    accumulation stays fp32 in PSUM.
"""
import sys

sys.path.insert(0, '/opt/trn_rl_repo')

import numpy as np

import concourse.bacc as bacc
import concourse.mybir as mybir
import concourse.tile as tile
from concourse.bass_utils import run_bass_kernel_spmd

F32 = mybir.dt.float32
BF16 = mybir.dt.bfloat16
U8 = mybir.dt.uint8
AF = mybir.ActivationFunctionType
ALU = mybir.AluOpType

B = 2
NQ = 512          # query tokens (i)
NCTX = 16384      # context tokens (j)
DM = 512          # model dim
HEADS = 8
DH = 64
INNER = 512
N_CORES = 8

KC = 4              # d_model chunks of 128
NJT = NCTX // 128   # 128 j-tiles
JCH = 2048          # context j-chunk per DMA (4 MiB fp32 source)
NCH = NCTX // JCH
VT = JCH // 128     # v tiles per j-chunk
SCALE = DH ** -0.5
MASK_BIG = 30000.0


def build_nc():
    nc = bacc.Bacc(None, target_bir_lowering=False, debug=False, num_devices=N_CORES)

    xt_d = nc.dram_tensor("xT", [DM, NQ], F32, kind="ExternalInput")
    ctxt_d = nc.dram_tensor("ctxT", [DM, NCTX], F32, kind="ExternalInput")
    msk_d = nc.dram_tensor("maskt", [128, NJT], U8, kind="ExternalInput")
    wq_d = nc.dram_tensor("wq", [DM, 128], F32, kind="ExternalInput")
    wk_d = nc.dram_tensor("wk", [DM, 128], F32, kind="ExternalInput")
    wv_d = nc.dram_tensor("wv", [DM, 128], F32, kind="ExternalInput")
    wout_d = nc.dram_tensor("wout", [INNER, INNER], F32, kind="ExternalInput")
    bout_d = nc.dram_tensor("boutr", [128, 4], F32, kind="ExternalInput")
    out_d = nc.dram_tensor("outT", [INNER, NQ], F32, kind="ExternalOutput")

    with tile.TileContext(nc) as tc:
        with (
            tc.tile_pool(name="const", bufs=1) as cpool,
            tc.tile_pool(name="big", bufs=1) as big,
            tc.tile_pool(name="ctx", bufs=4) as ctxpool,
            tc.tile_pool(name="pt", bufs=3) as ptpool,
            tc.tile_pool(name="fin", bufs=2) as fin,
            tc.tile_pool(name="ps", bufs=3, space="PSUM") as pps,
            tc.tile_pool(name="av", bufs=1, space="PSUM") as pav,
            tc.tile_pool(name="dram", bufs=1, space="DRAM") as dram,
        ):
            # ---- weights / x / mask: fp32 staging on the HWDGE queue, DVE cast.
            # (keeps the gpsimd SWDGE queue free for the big context stream)
            msk_u8 = cpool.tile([128, NJT], U8)
            nc.sync.dma_start(out=msk_u8[:], in_=msk_d[:, :])
            bout_sb = cpool.tile([128, 4], F32)
            nc.sync.dma_start(out=bout_sb[:], in_=bout_d[:, :])
            wq_f = cpool.tile([128, KC, 128], F32)
            wk_f = cpool.tile([128, KC, 128], F32)
            wv_f = cpool.tile([128, KC, 128], F32)
            xt_f = cpool.tile([128, KC, NQ], F32)
            wout_f = cpool.tile([128, KC, INNER], F32)
            wq_bf = cpool.tile([128, KC, 128], BF16)
            wk_bf = cpool.tile([128, KC, 128], BF16)
            wv_bf = cpool.tile([128, KC, 128], BF16)
            xt_bf = cpool.tile([128, KC, NQ], BF16)
            wout_bf = cpool.tile([128, KC, INNER], BF16)
            for dst, stage, src in ((wq_bf, wq_f, wq_d), (xt_bf, xt_f, xt_d),
                                    (wk_bf, wk_f, wk_d), (wv_bf, wv_f, wv_d),
                                    (wout_bf, wout_f, wout_d)):
                nc.sync.dma_start(
                    out=stage[:], in_=src.ap().rearrange("(k p) n -> p k n", p=128))
                nc.vector.tensor_copy(dst[:], stage[:])

            msk_f = cpool.tile([128, NJT], F32)
            nc.vector.tensor_copy(msk_f[:], msk_u8[:])
            bias_sb = cpool.tile([128, NJT], F32)
            nc.vector.tensor_scalar(bias_sb[:], msk_f[:], MASK_BIG, -MASK_BIG,
                                    ALU.mult, ALU.add)

            # ones row lives at partition 64 so it can pair with row-64 slices
            ones_sb = cpool.tile([65, 65], F32)
            nc.vector.memset(ones_sb[64:65, :], 1.0)

            # ---- qT: [128 (2h x 64d), NQ] bf16 ----
            psum_q = pps.tile([128, NQ], F32, tag="ps")
            for k in range(KC):
                nc.tensor.matmul(psum_q[:], wq_bf[:, k, :], xt_bf[:, k, :],
                                 start=(k == 0), stop=(k == KC - 1))
            qt_bf = cpool.tile([128, NQ], BF16)
            nc.vector.tensor_copy(qt_bf[:], psum_q[:])

            # ---- kT [128 (2h x 64d), NCTX] and v_aug [128 (j), NJT, 130] ----
            kt_bf = big.tile([128, NCTX], BF16)
            # per j-tile: [vA(64) | onesA | vB(64) | onesB] so the AV matmul's
            # ones-column accumulates the softmax denominator into row 64
            v_bf = big.tile([128, NJT, 130], BF16)
            nc.vector.memset(v_bf[:, :, 64:65], 1.0)
            nc.vector.memset(v_bf[:, :, 129:130], 1.0)

            def kv_piece(j0, width):
                ctx_bf = ctxpool.tile([128, KC, width], BF16, tag="ctx",
                                      name=f"ctx_{j0}")
                nc.gpsimd.dma_start(
                    out=ctx_bf[:],
                    in_=ctxt_d.ap()[:, j0:j0 + width].rearrange("(k p) j -> p k j", p=128))
                for s in range(width // 512):
                    psum_kt = pps.tile([128, 512], F32, tag="ps", name=f"pkt_{j0}_{s}")
                    for k in range(KC):
                        nc.tensor.matmul(psum_kt[:], wk_bf[:, k, :],
                                         ctx_bf[:, k, s * 512:(s + 1) * 512],
                                         start=(k == 0), stop=(k == KC - 1))
                    nc.vector.tensor_copy(
                        kt_bf[:, j0 + s * 512:j0 + (s + 1) * 512], psum_kt[:])
                for t in range(width // 128):
                    jt = j0 // 128 + t
                    psum_v = pps.tile([128, 128], F32, tag="ps", name=f"pv_{jt}")
                    for k in range(KC):
                        nc.tensor.matmul(psum_v[:], ctx_bf[:, k, t * 128:(t + 1) * 128],
                                         wv_bf[:, k, :], start=(k == 0), stop=(k == KC - 1))
                    nc.vector.tensor_copy(v_bf[:, jt, 0:64], psum_v[:, 0:64])
                    nc.vector.tensor_copy(v_bf[:, jt, 65:129], psum_v[:, 64:128])

            for j0 in range(0, JCH, 512):       # warm-up: small pieces
                kv_piece(j0, 512)
            for c in range(1, NCH):
                kv_piece(c * JCH, JCH)

            # ---- attention: simT -> exp -> AV accumulate ----
            psum_av = [pav.tile([65, NQ], F32, tag=f"av{h}", name=f"psum_av{h}")
                       for h in range(2)]
            for t in range(NJT):
                psum_s = pps.tile([128, 2 * NQ], F32, tag="ps")
                for h in range(2):
                    nc.tensor.matmul(psum_s[:, h * NQ:(h + 1) * NQ],
                                     kt_bf[h * 64:(h + 1) * 64, t * 128:(t + 1) * 128],
                                     qt_bf[h * 64:(h + 1) * 64, :],
                                     start=True, stop=True)
                pt_bf = ptpool.tile([128, 2 * NQ], BF16, tag="pt")
                nc.scalar.activation(pt_bf[:], psum_s[:], AF.Exp,
                                     bias=bias_sb[:, t:t + 1], scale=SCALE)
                for h in range(2):
                    nc.tensor.matmul(psum_av[h][:],
                                     v_bf[:, t, h * 65:(h + 1) * 65],
                                     pt_bf[:, h * NQ:(h + 1) * NQ],
                                     start=(t == 0), stop=(t == NJT - 1),
                                     skip_group_check=True)

            # ---- normalize by the softmax denominator, stage for AllGather ----
            ag_in = dram.tile([128, NQ], BF16)
            ag_out = dram.tile([INNER, NQ], BF16)
            for h in range(2):
                linv = fin.tile([65, NQ], F32, tag="linv")
                nc.vector.reciprocal(linv[64:65, :], psum_av[h][64:65, :])
                psum_lb = pps.tile([65, NQ], F32, tag="ps")
                nc.tensor.matmul(psum_lb[:], ones_sb[64:65, :], linv[64:65, :],
                                 start=True, stop=True)
                linvb = fin.tile([65, NQ], F32, tag="linvb")
                nc.vector.tensor_copy(linvb[:], psum_lb[:])
                avn = fin.tile([64, NQ], BF16, tag="avn")
                nc.vector.tensor_tensor(avn[:], psum_av[h][0:64, :],
                                        linvb[0:64, :], ALU.mult)
                nc.sync.dma_start(out=ag_in[h * 64:(h + 1) * 64, :], in_=avn[:])

            nc.gpsimd.collective_compute(
                "AllGather", ALU.bypass,
                replica_groups=[[0, 1, 2, 3], [4, 5, 6, 7]],
                ins=[ag_in[:].opt()],
                outs=[ag_out[:].opt()],
            )

            # ---- output projection: outT = Wout^T @ attnoutT (+ bout) ----
            att_bf = cpool.tile([128, KC, NQ], BF16)
            nc.sync.dma_start(
                out=att_bf[:], in_=ag_out[:].rearrange("(k p) n -> p k n", p=128))
            for m in range(4):
                psum_o = pps.tile([128, NQ], F32, tag="ps")
                for k in range(KC):
                    nc.tensor.matmul(psum_o[:], wout_bf[:, k, m * 128:(m + 1) * 128],
                                     att_bf[:, k, :], start=(k == 0), stop=(k == KC - 1))
                out_sb = fin.tile([128, NQ], F32, tag="out")
                nc.scalar.activation(out_sb[:], psum_o[:], AF.Identity,
                                     bias=bout_sb[:, m:m + 1])
                nc.sync.dma_start(out=out_d[m * 128:(m + 1) * 128, :], in_=out_sb[:])

    nc.compile()
    return nc


_NC = None


def _get_nc():
    global _NC
    if _NC is None:
        _NC = build_nc()
    return _NC


def kernel(x, context, mask, Wq, Wkv, Wout, bout):
    x = np.asarray(x, dtype=np.float32)
    context = np.asarray(context, dtype=np.float32)
    mask = np.asarray(mask)
    Wq = np.asarray(Wq, dtype=np.float32)
    Wkv = np.asarray(Wkv, dtype=np.float32)
    Wout = np.asarray(Wout, dtype=np.float32)
    bout = np.asarray(bout, dtype=np.float32)

    nc = _get_nc()
    in_maps = []
    for core in range(N_CORES):
        b, hg = core // 4, core % 4
        cs = slice(hg * 128, (hg + 1) * 128)
        in_maps.append({
            "xT": np.ascontiguousarray(x[b].T),
            "ctxT": np.ascontiguousarray(context[b].T),
            "maskt": np.ascontiguousarray(
                mask[b].reshape(NJT, 128).T.astype(np.uint8)),
            "wq": np.ascontiguousarray(Wq[:, cs]),
            "wk": np.ascontiguousarray(Wkv[:, :INNER][:, cs]),
            "wv": np.ascontiguousarray(Wkv[:, INNER:][:, cs]),
            "wout": Wout,
            "boutr": np.ascontiguousarray(bout.reshape(4, 128).T),
        })

    res = run_bass_kernel_spmd(nc, in_maps, list(range(N_CORES)))
    out = np.empty((B, NQ, INNER), dtype=np.float32)
    for b in range(B):
        out[b] = res.results[4 * b]["outT"].T
    return out
